# revision 16
# baseline (speedup 1.0000x reference)
"""DGCNN (nn_DGCNN_43911745634410) Trainium2 kernel.

Structure of the model: the only heavy compute is xw = x @ gcn1_W with
x [129, 262144] f32 (~135 MB) and gcn1_W [262144, 1] — a memory-bound matvec.
xw is shared by all three edge-attr channels (it does not depend on edge
weights). Everything downstream (segment-sums over 16K edges, a 129-element
sort, two tiny conv1ds and three FCs) is a few hundred KFLOPs.

Device strategy (8 NeuronCores, tensor-parallel over the feature dim F):
  - core c gets x[:, c*32768:(c+1)*32768] (16.5 MB) and the matching w slice;
  - a raw-Bass kernel streams the shard through SBUF and uses the DVE's fused
    scalar_tensor_tensor (out=(x*1)*w, accum_out=free-dim sum) to produce
    per-partition partial dot products at one DVE pass per element, so the
    kernel runs at the HBM/DMA roofline (~47 us per core);
  - bulk tiles are [128, 1024] (4 rows x 32 partitions-per-row), the last row
    is one short [128, 256] tile so the non-overlapped tail op is short.
  - partials ([128, 33] per core) are summed on the host in f64 (all-reduce
    across cores), and the tiny downstream runs on the host in f64, exactly
    matching the reference semantics (stable descending sort, PyG GCN
    normalization with self-loops, VALID conv1d/maxpool, ELU MLP).

The raw-Bass (no TileContext) form is deliberate: this toolchain encodes at
most ONE semaphore wait per instruction, so each x tile gets a dedicated SBUF
buffer (the whole shard fits: ~132 KB/partition of the 224 KB) and every wait
is a single explicit wait_ge.
"""
from contextlib import ExitStack

import numpy as np

import concourse.bass as bass
from concourse import mybir
from concourse.bass_utils import run_bass_kernel_spmd

F32 = mybir.dt.float32
I16 = mybir.dt.int16

N = 129
F = 262144
NCORES = 8
SH = F // NCORES          # 32768 features per core
FD = 1024                 # free elems per partition per bulk tile
PPR = SH // FD            # partitions per row = 32
RPT = 128 // PPR          # rows per bulk tile = 4
NB = 26                   # bulk tiles [128, 1024], rows 0..103
TFD = SH // 128           # 256: small-tile free dim (one row per tile)
NS = 25                   # small tiles [128, 256], rows 104..128
NCOL = NB + NS            # 51 partial columns
NC1 = NCOL - 3            # columns in the early output DMA

_NC_CACHE = None


def _build_matvec_bass():
    nc = bass.Bass("TRN2")
    x = nc.dram_tensor("x_s", [N * SH], F32, kind="ExternalInput")
    w = nc.dram_tensor("w_s", [SH], F32, kind="ExternalInput")
    out = nc.dram_tensor("part", [128, NC1], F32, kind="ExternalOutput")
    # plast rows are padded to 64 f32: the scatter-add DGE requires a
    # 256-byte DRAM row stride.
    outl = nc.dram_tensor("plast", [128, 64], F32, kind="ExternalOutput")

    with ExitStack() as ctx:
        selt = ctx.enter_context(nc.sbuf_tensor("selt", [32, 259], F32))
        ones = ctx.enter_context(nc.sbuf_tensor("ones", [32, 128], F32))
        idxt = ctx.enter_context(nc.sbuf_tensor("idxt", [128, 8], I16))
        idtmp = ctx.enter_context(nc.sbuf_tensor("idtmp", [128, 8], I16))
        zbuf = ctx.enter_context(nc.sbuf_tensor("zbuf", [128, NCOL - NC1], F32))
        wq = ctx.enter_context(nc.sbuf_tensor("wq", [32, FD], F32))
        wnt = ctx.enter_context(nc.sbuf_tensor("wnt", [128, TFD], F32))
        wt_ps = ctx.enter_context(nc.psum_tensor("wt_ps", [128, FD], F32))
        wn_ps = ctx.enter_context(nc.psum_tensor("wn_ps", [128, TFD], F32))
        wt_sb = ctx.enter_context(nc.sbuf_tensor("wt_sb", [128, FD], F32))
        xts = [
            ctx.enter_context(nc.sbuf_tensor(f"xt{t}", [128, FD], F32))
            for t in range(NB)
        ]
        xss = [
            ctx.enter_context(nc.sbuf_tensor(f"xs{s}", [128, TFD], F32))
            for s in range(NS)
        ]
        part = ctx.enter_context(nc.sbuf_tensor("part_sb", [128, NCOL], F32))
        w_sem = ctx.enter_context(nc.semaphore("w_sem"))
        sel_sem = ctx.enter_context(nc.semaphore("sel_sem"))
        g_sem = ctx.enter_context(nc.semaphore("g_sem"))
        z_sem = ctx.enter_context(nc.semaphore("z_sem"))
        sc_sem = ctx.enter_context(nc.semaphore("sc_sem"))
        o1_sem = ctx.enter_context(nc.semaphore("o1_sem"))
        pe_sem = ctx.enter_context(nc.semaphore("pe_sem"))
        act_sem = ctx.enter_context(nc.semaphore("act_sem"))
        # All x DMAs are issued from SP's single HWDGE ring, so they
        # complete in issue order: one semaphore with cumulative thresholds
        # replaces the per-tile semaphores (fewer sems -> one init memset
        # at program entry instead of four, an earlier stream start).
        x_sem = ctx.enter_context(nc.semaphore("x_sem"))
        s_sem = ctx.enter_context(nc.semaphore("s_sem"))
        dve_sem = ctx.enter_context(nc.semaphore("dve_sem"))
        block = ctx.enter_context(nc.Block())

        base = NB * 128 * FD

        @block.sync
        def _(sync):
            # Tile 0 is loaded by the gpsimd prepared-gather, whose SWDGE
            # descriptor-gen + trigger beats SP's HWDGE first-DMA latency by
            # ~1 us; SP feeds everything after it, starting with wq.
            sync.dma_start(
                wq[:, :], w[:].rearrange("(q j) -> q j", j=FD)
            ).then_inc(w_sem, 16)
            for t in range(1, NB):
                src = x[t * 128 * FD : (t + 1) * 128 * FD].rearrange(
                    "(p f) -> p f", f=FD
                )
                sync.dma_start(xts[t][:, :], src).then_inc(x_sem, 16)
                if t == 2:
                    # zero plast so the end-of-kernel scatter-ADD lands on
                    # exact zeros; placed here so neither its wait nor its
                    # descriptor-gen perturbs the stream head.
                    sync.wait_ge(z_sem, 1)
                    sync.dma_start(
                        outl[:, 0 : NCOL - NC1], zbuf[:, :]
                    ).then_inc(z_sem, 16)
            for s in range(NS):
                src = x[base + s * 128 * TFD : base + (s + 1) * 128 * TFD].rearrange(
                    "(p f) -> p f", f=TFD
                )
                sync.dma_start(xss[s][:, :], src).then_inc(s_sem, 16)
            # Columns 0..NC1 go out as soon as their STTs are done: this
            # DMA's SEQ/descriptor-gen/DGE latency and its completion-sem
            # propagation all hide under the stream tail. The last three
            # columns are written by the gpsimd prepared scatter-add, whose
            # trigger skips the HWDGE gen + DGE->DMA latency entirely.
            sync.wait_ge(dve_sem, NC1)
            sync.dma_start(out[:, :], part[:, 0:NC1]).then_inc(o1_sem, 16)

        @block.gpsimd
        def _(gpsimd):
            # Identity gather/scatter indices, int16, wrapped in 16
            # partitions: idx[j] lives at [j % 16, j // 16], so
            # idxt[p, s] = p + 16*s for p < 16. The DGE asserts every value
            # (all 128 partitions) is in [-1, n_rows), so mask partitions
            # >= 16 to 0 with an affine_select on p.
            gpsimd.iota(
                idtmp[:, :], pattern=[[16, 8]], base=0, channel_multiplier=1
            )
            gpsimd.affine_select(
                idxt[:, :], idtmp[:, :],
                pattern=[[0, 8]],
                compare_op=mybir.AluOpType.is_ge,
                fill=0.0, base=15, channel_multiplier=-1,
            )
            # Tile 0 via prepared gather: descriptors are generated here
            # (SWDGE), the trigger fires them with no DGE->DMA delay, so the
            # first x transfer starts ~1 us before the HWDGE path could.
            gpsimd.dma_gather(
                xts[0][:, :].rearrange("p (c f) -> p c f", c=1),
                x[0 : 128 * FD].rearrange("(p f) -> p f", f=FD),
                idxt[:, :],
                num_idxs=128, num_idxs_reg=128, elem_size=FD,
                prepare_only=True, sem=g_sem,
            )
            gpsimd.trigger_dma(count=1)
            gpsimd.memset(zbuf[:, :], 0.0).then_inc(z_sem, 1)
            # Build the PE selector on-chip (keeps it out of the DMA stream):
            #   selt[:, 0:128]   = tile(eye(32), (1, 4))   (wt broadcast)
            #   selt[:, 128:131] = 0                       (guard band)
            #   selt[:, 131:259] : selt[q, 131+4q] = 1     (wnt permutation;
            #     matmuls read selt[:, 131-b : 259-b] for b in 0..3)
            gpsimd.memset(ones[:, :], 1.0)
            gpsimd.memset(selt[:, 128:131], 0.0)
            gpsimd.affine_select(
                selt[:, 0:128], ones[:, :],
                pattern=[[0, 4], [1, 32]],
                compare_op=mybir.AluOpType.is_equal,
                fill=0.0, base=0, channel_multiplier=-1,
            )
            gpsimd.affine_select(
                selt[:, 131:259], ones[:, :],
                pattern=[[1, 128]],
                compare_op=mybir.AluOpType.is_equal,
                fill=0.0, base=0, channel_multiplier=-4,
            ).then_inc(sel_sem, 1)
            # Final output columns via prepared scatter-add: descriptors are
            # ready long before the last STT; the trigger costs only the Pool
            # SEQ dispatch + the 56 ns transfer, vs ~1.3 us for a fresh HWDGE
            # DMA (SEQ + descriptor-gen + DGE->DMA delay).
            gpsimd.dma_scatter_add(
                outl[:, 0 : NCOL - NC1],
                part[:, NC1:NCOL].rearrange("p (c f) -> p c f", c=1),
                idxt[:, :],
                num_idxs=128, num_idxs_reg=128, elem_size=NCOL - NC1,
                elem_step=64,
                prepare_only=True, sem=sc_sem,
            )
            gpsimd.wait_ge(dve_sem, NCOL)
            gpsimd.trigger_dma(count=1)

        @block.tensor
        def _(tensor):
            tensor.wait_ge(sel_sem, 1)
            tensor.wait_ge(w_sem, 16)
            nc.tensor.matmul(
                wt_ps[:, 0:512], selt[:, 0:128], wq[:, 0:512],
                start=True, stop=True,
            ).then_inc(pe_sem, 1)
            nc.tensor.matmul(
                wt_ps[:, 512:FD], selt[:, 0:128], wq[:, 512:FD],
                start=True, stop=True,
            ).then_inc(pe_sem, 1)
            # wn_ps[p, i] = wq[p//4, (p%4)*256 + i]: four accumulating
            # matmuls; lhsT_b = iselt[:, 3-b : 131-b] has ones at (q, 4q+b),
            # so pass b contributes rows p%4 == b and exact zeros elsewhere.
            for b in range(4):
                nc.tensor.matmul(
                    wn_ps[:, :], selt[:, 131 - b : 259 - b],
                    wq[:, b * TFD : (b + 1) * TFD],
                    start=(b == 0), stop=(b == 3),
                ).then_inc(pe_sem, 1)

        @block.scalar
        def _(scalar):
            scalar.wait_ge(pe_sem, 2)
            nc.scalar.copy(wt_sb[:, :], wt_ps[:, :]).then_inc(act_sem, 1)
            scalar.wait_ge(pe_sem, 6)
            nc.scalar.copy(wnt[:, :], wn_ps[:, :]).then_inc(act_sem, 1)

        @block.vector
        def _(vector):
            vector.wait_ge(act_sem, 1)
            for t in range(NB):
                if t == 0:
                    vector.wait_ge(g_sem, 16)
                else:
                    vector.wait_ge(x_sem, 16 * t)
                nc.vector.scalar_tensor_tensor(
                    xts[t][:, :],
                    xts[t][:, :],
                    1.0,
                    wt_sb[:, :],
                    op0=mybir.AluOpType.mult,
                    op1=mybir.AluOpType.mult,
                    accum_out=part[:, t : t + 1],
                ).then_inc(dve_sem, 1)
            vector.wait_ge(act_sem, 2)
            for s in range(NS):
                vector.wait_ge(s_sem, 16 * (s + 1))
                nc.vector.scalar_tensor_tensor(
                    xss[s][:, :],
                    xss[s][:, :],
                    1.0,
                    wnt[:, :],
                    op0=mybir.AluOpType.mult,
                    op1=mybir.AluOpType.mult,
                    accum_out=part[:, NB + s : NB + s + 1],
                ).then_inc(dve_sem, 1)

    # Populate .instr bytes for raw-ISA instructions (InstTriggerDma): the
    # Bacc compile flow does this, the plain Bass path does not, and walrus
    # rejects a zero-length ISA payload.
    mybir.codegen_inst_isa_subclasses(nc)
    return nc



def get_matvec_bass():
    global _NC_CACHE
    if _NC_CACHE is None:
        _NC_CACHE = _build_matvec_bass()
    return _NC_CACHE


def _make_core_inputs(x_np, w_np, core):
    xs = np.ascontiguousarray(x_np[:, core * SH : (core + 1) * SH]).reshape(-1)
    ws = np.ascontiguousarray(w_np[core * SH : (core + 1) * SH])
    return {"x_s": xs, "w_s": ws}


def _reduce_parts(parts):
    """parts: 8 pairs ([128, NC1], [128, NCOL-NC1]) f32 -> xw [N] f64."""
    xw = np.zeros(N, np.float64)
    for pb, pl in parts:
        p = np.concatenate([pb, pl[:, 0 : NCOL - NC1]], axis=1).astype(np.float64)
        for t in range(NB):
            xw[RPT * t : RPT * (t + 1)] += p[:, t].reshape(RPT, PPR).sum(1)
        for si in range(NS):
            xw[RPT * NB + si] += p[:, NB + si].sum()
    return xw


def _matvec_device(x_np, w_np):
    """x [N, F] f32, w [F] f32 -> xw [N] f64 via the 8-core bass kernel."""
    global _NC_CACHE
    in_maps = [_make_core_inputs(x_np, w_np, c) for c in range(NCORES)]
    last_exc = None
    for attempt in range(2):
        try:
            nc = get_matvec_bass()
            res = run_bass_kernel_spmd(nc, in_maps, core_ids=list(range(NCORES)))
            return _reduce_parts(
                [(res.results[c]["part"], res.results[c]["plast"])
                 for c in range(NCORES)]
            )
        except Exception as e:  # transient NRT_EXEC_UNIT_UNRECOVERABLE seen once
            import sys

            print(f"kernel: device run attempt {attempt} failed: {e!r:.200}",
                  file=sys.stderr)
            last_exc = e
            _NC_CACHE = None
    # Last-resort host fallback so a transient device failure still yields a
    # correct result (numerically equivalent partial-sum structure).
    import sys

    print(f"kernel: device path failed twice ({last_exc!r:.200}); "
          "falling back to host matvec", file=sys.stderr)
    prod = x_np.astype(np.float64) * w_np.astype(np.float64)[None, :]
    return prod.sum(axis=1)


def _downstream(xw, inputs):
    """Everything after xw = x @ gcn1_W, in f64 numpy. Returns [1, 2] f32."""
    edge_index = np.asarray(inputs["edge_index"]).astype(np.int64)
    row, col = edge_index[0], edge_index[1]
    edge_attr = np.asarray(inputs["edge_attr"], np.float64)
    g1b = np.asarray(inputs["gcn1_b"], np.float64)
    g2W = np.asarray(inputs["gcn2_W"], np.float64)
    g2b = np.asarray(inputs["gcn2_b"], np.float64)
    c1w = np.asarray(inputs["conv1_w"], np.float64)
    c1b = np.asarray(inputs["conv1_b"], np.float64)
    c2w = np.asarray(inputs["conv2_w"], np.float64)
    c2b = np.asarray(inputs["conv2_b"], np.float64)
    f1W = np.asarray(inputs["fc1_W"], np.float64)
    f1b = np.asarray(inputs["fc1_b"], np.float64)
    f2W = np.asarray(inputs["fc2_W"], np.float64)
    f2b = np.asarray(inputs["fc2_b"], np.float64)
    f3W = np.asarray(inputs["fc3_W"], np.float64)
    f3b = np.asarray(inputs["fc3_b"], np.float64)

    n = N
    loop = np.arange(n)
    row2 = np.concatenate([row, loop])
    col2 = np.concatenate([col, loop])

    def gcn(xw_vec, ew):
        # PyG GCNConv with edge weights: self-loops (weight 1), symmetric norm.
        ew2 = np.concatenate([ew, np.ones(n)])
        deg = np.zeros(n)
        np.add.at(deg, col2, ew2)
        dinv = np.where(deg > 0, deg**-0.5, 0.0)
        norm = dinv[row2] * ew2 * dinv[col2]
        out = np.zeros(n)
        np.add.at(out, col2, norm * xw_vec[row2])
        return out

    outs = []
    for c in range(3):
        ew = edge_attr[:, c]
        h1 = gcn(xw, ew) + g1b[0]
        h2 = gcn(h1 * g2W[0, 0], ew) + g2b[0]
        # SortPool: jnp.argsort(-h2) is a stable ascending sort of the negation
        perm = np.argsort(-h2, kind="stable")
        hs = np.stack([h1[perm], h2[perm]], axis=1)  # [n, 2]
        z = hs.T  # [2, n]
        L = z.shape[1] - 2
        z1 = np.zeros((3, L))
        for o in range(3):
            for i in range(2):
                for k in range(3):
                    z1[o] += c1w[o, i, k] * z[i, k : k + L]
            z1[o] += c1b[o]
        z1p = np.max(np.stack([z1[:, 0 : L - 2], z1[:, 1 : L - 1], z1[:, 2:L]], 0), 0)
        L2 = z1p.shape[1] - 2
        z2 = np.zeros((1, L2))
        for i in range(3):
            for k in range(3):
                z2[0] += c2w[0, i, k] * z1p[i, k : k + L2]
        z2[0] += c2b[0]
        z2p = np.max(
            np.stack([z2[:, 0 : L2 - 2], z2[:, 1 : L2 - 1], z2[:, 2:L2]], 0), 0
        )
        outs.append(z2p)  # [1, 121]

    allx = np.concatenate(outs, axis=0)  # [3, 121]
    h = allx.reshape(1, -1)

    def elu(v):
        return np.where(v > 0, v, np.expm1(v))

    h = elu(h @ f1W + f1b)
    h = elu(h @ f2W + f2b)
    out = h @ f3W + f3b
    return out.astype(np.float32)


def kernel(**inputs) -> np.ndarray:
    x = np.ascontiguousarray(np.asarray(inputs["x"], np.float32))
    w = np.asarray(inputs["gcn1_W"], np.float32).reshape(-1)
    xw = _matvec_device(x, w)
    return _downstream(xw, inputs)

